# revision 23
# baseline (speedup 1.0000x reference)
"""Trainium2 Bass kernel for nn_ContrastiveLoss_rec (8-core data-parallel).

Math (per reference):
    wA_is = A_is @ W.T + b ; wA_em = A_em @ W.T + b
    diag_is = sum((0.4*m + 0.6*tr_m) * wA_is, -1)
    diag_em = sum((0.4*m + 0.6*tr_m) * wA_em, -1)
    loss = sum(max(0.2 + diag_is - diag_em, 0))

Algebraic simplification used here:
    mc  = 0.4*m + 0.6*tr_m          (bias b cancels in the difference)
    z   = rowdot(mc, (A_is - A_em) @ W.T)
        = rowdot(D, mc @ W)          with D = A_is - A_em
    loss = sum(max(0.2 + z, 0))
Folding the 0.6:  mc = 0.6*(tr_m + (2/3) m) = 0.6*mc'
    loss = 0.6 * sum(max(z' + 1/3, 0)),  z' = rowdot(D, mc' @ W)

Per-core plan (B_loc = 1024 rows), pipelined for continuous DMA + warm PE:
  - 1 MiB HWDGE DMAs (st=2 b-tiles per supertile) alternating the sync /
    scalar rings; "(p t) e" row mapping gives 8 KiB contiguous per
    partition line. m/tr_m/W are declared f32r at DRAM (same bits, PE
    fp32r path, no cast DMA needed).
  - W loads in 1 MiB k-chunk DMAs riding behind supertile-0's inputs;
    next rep's W prefetches into a 2-deep ring during this rep's tail
    supertiles so rep boundaries never stall on the 4 MiB W blob.
  - D = A_is - A_em on GPSIMD (off the DVE critical path);
    mc' = (2/3) m + tr_m in one DVE scalar_tensor_tensor.
  - PE transposes mc' chunks via f32r identity (1.5 cyc/row); ACT copies
    PSUM->SBUF; main matmul P = mc' @ W in f32r (full rate, 1 cyc/row).
  - DVE fused rowdot with accum: z' partial per (tile, n-chunk), emitted
    AFTER the next tile's DVE prep (software pipelining) so DVE's
    in-order queue never blocks the next tile's transposes.
  - Hinge + row reduce, partition reduce via matmul with a 0.6-filled
    ones vector, scalar out per core; host sums the 8 partials.
"""

import numpy as np

import concourse.bass as bass
import concourse.mybir as mybir
import concourse.tile as tile
from concourse.bass_utils import run_bass_kernel_spmd

N_CORES = 8
B, E = 8192, 1024
B_LOC = B // N_CORES          # 1024 rows per core
P = 128                       # partitions
NBT = B_LOC // P              # 8 b-tiles per core
ST = 2                        # b-tiles per DMA super-tile (1 MiB DMAs)
NST = NBT // ST               # 4 super-tiles
KT = E // P                   # 8 contraction chunks
NF = 512                      # matmul moving free dim (one PSUM bank fp32)
NCH = E // NF                 # 2 n-chunks

F32 = mybir.dt.float32
F32R = mybir.dt.float32r
AX = mybir.AluOpType


def _make_scaled_identity(nc, ap, val, scratch=None):
    """Build val*I. For f32r targets, build in an f32 scratch then round in
    via tensor_copy (the BIR verifier requires f32r matmul operands to come
    from a rounding producer)."""
    tgt = ap if scratch is None else scratch
    nc.gpsimd.memset(tgt, 0.0)
    nc.gpsimd.affine_select(
        out=tgt,
        in_=tgt,
        compare_op=AX.not_equal,
        fill=float(val),
        base=0,
        pattern=[[-1, tgt.shape[1]]],
        channel_multiplier=1,
    )
    if scratch is not None:
        nc.vector.tensor_copy(ap, tgt)


def build(st=2, io_bufs=3, dma_engines=("sync", "scalar"), repeat=1, w_bufs=2,
          act_dt="f32r", w_chunk=2, t_dt="f32r", mm_dt="f32r"):
    """Build the single-core Bass program (SPMD across 8 cores)."""
    nst = NBT // st
    DT = F32R if act_dt == "f32r" else mybir.dt.bfloat16
    TDT = F32R if t_dt == "f32r" else mybir.dt.bfloat16
    FP8 = mm_dt == "fp8"
    F8 = mybir.dt.float8e4
    cast = act_dt != "f32r"
    nc = bass.Bass(
        "TRN2", target_bir_lowering=False, debug=False, num_devices=N_CORES
    )

    # m / tr_m / W are declared float32r at DRAM: same 32-bit container
    # (host feeds fp32 bits), PE applies its internal fp32r rounding. This
    # lets W load on HWDGE (no SWDGE cast) and makes the transposes 1.5
    # cycles/row instead of 2.0.
    A_is = nc.dram_tensor("a_is", [B_LOC, E], F32, kind="ExternalInput").ap()
    A_em = nc.dram_tensor("a_em", [B_LOC, E], F32, kind="ExternalInput").ap()
    m_dram_dt = F32 if cast else F32R
    M_in = nc.dram_tensor("m_in", [B_LOC, E], m_dram_dt, kind="ExternalInput").ap()
    TR_m = nc.dram_tensor("tr_m", [B_LOC, E], m_dram_dt, kind="ExternalInput").ap()
    W_in = nc.dram_tensor("w_in", [E, E], m_dram_dt, kind="ExternalInput").ap()
    OUT = nc.dram_tensor("out", [1, 1], F32, kind="ExternalOutput").ap()

    _dma_idx = [0]

    def dma(dst, src):
        if dst.dtype != src.dtype:
            nc.gpsimd.dma_start(dst, src)  # SWDGE casts during the load
            return
        eng = getattr(nc, dma_engines[_dma_idx[0] % len(dma_engines)])
        _dma_idx[0] += 1
        eng.dma_start(dst, src)

    with tile.TileContext(nc) as tc:
        with (
            tc.tile_pool(name="const", bufs=1) as cpool,
            tc.tile_pool(name="wpool", bufs=1) as wpool,
            tc.tile_pool(name="w8pool", bufs=1) as w8pool,
            tc.tile_pool(name="wstage", bufs=3) as wstage,
            tc.tile_pool(name="io", bufs=io_bufs) as iopool,
            tc.tile_pool(name="dbuf", bufs=2) as dpool,
            tc.tile_pool(name="mct", bufs=2) as mctpool,
            tc.tile_pool(name="ttr", bufs=2) as ttrpool,
            tc.tile_pool(name="acc", bufs=1) as accpool,
            tc.tile_pool(name="ps_t", bufs=2, space="PSUM") as pst,
            tc.tile_pool(name="ps_mm", bufs=4, space="PSUM") as psmm,
            tc.tile_pool(name="ps_fin", bufs=1, space="PSUM") as psfin,
        ):
            # W chunk loads: per-rep ring (w_bufs deep); chunks are
            # interleaved into the input stream by the caller below.
            def w_tile():
                if not FP8:
                    w = wpool.tile(
                        [P, KT, E], DT, tag="w", bufs=w_bufs, name="w_sb"
                    )
                    return w, w
                w8 = w8pool.tile(
                    [P, KT, E], F8, tag="w8", bufs=w_bufs, name="w8_sb"
                )
                return None, w8

            W_src = W_in.rearrange("(ko p) n -> p ko n", p=P)

            def load_w_chunks(wpair, k0, k1, wc=None):
                w_sb, w8_sb = wpair
                wc = wc or w_chunk
                for kk in range(k0, k1, wc):
                    n = min(wc, k1 - kk)
                    if FP8:
                        # stage fp32r chunk, quantize to fp8 on ACT
                        stg = wstage.tile(
                            [P, wc, E], DT, tag="wst", name="wst"
                        )
                        dma(
                            stg[:, bass.ds(0, n), :],
                            W_src[:, bass.ds(kk, n), :],
                        )
                        nc.scalar.copy(
                            w8_sb[:, bass.ds(kk, n), :], stg[:, bass.ds(0, n), :]
                        )
                    else:
                        dma(
                            w_sb[:, bass.ds(kk, n), :],
                            W_src[:, bass.ds(kk, n), :],
                        )

            ident_f32 = cpool.tile([P, P], F32)
            ident1 = cpool.tile([P, P], TDT)
            _make_scaled_identity(nc, ident1[:], 1.0, scratch=ident_f32[:])
            ones06 = cpool.tile([P, 1], F32)
            nc.vector.memset(ones06[:], 0.6)

            w_cur = w_tile()
            for _rep in range(repeat):
                # z' partials: one column per (b-tile, n-chunk)
                zacc = accpool.tile([P, NBT * NCH], F32, tag="zacc")
                w_pair = w_cur
                w_sb = w_pair[1]
                w_next = None

                # pending rowdot from the previous tile: emitted AFTER the
                # next tile's DVE prep so DVE's in-order queue never blocks
                # the next tile's transposes behind a PSUM-dependent ttr.
                pending = []

                def emit_ttr():
                    for pm, dt_ap, zi in pending:
                        ttr_out = ttrpool.tile(
                            [P, NF], F32, tag="ttro", name="ttro"
                        )
                        nc.vector.scalar_tensor_tensor(
                            out=ttr_out[:],
                            in0=pm[:],
                            scalar=1.0,
                            in1=dt_ap,
                            op0=AX.mult,
                            op1=AX.mult,
                            accum_out=zacc[:, zi : zi + 1],
                        )
                    pending.clear()

                for s in range(nst):
                    rows = bass.ds(s * st * P, st * P)

                    # "(p t) e": each partition holds st consecutive HBM rows
                    # -> st*4 KiB contiguous per partition line, fewer/larger
                    # DMA descriptors. Row permutation is harmless: the final
                    # result is a sum over rows and all four tensors (and the
                    # per-tile rowdot) use the same mapping.
                    m_t = iopool.tile([P, st, E], DT, tag="m")
                    trm_t = iopool.tile([P, st, E], DT, tag="trm")
                    ais_t = iopool.tile([P, st, E], F32, tag="ais")
                    aem_t = iopool.tile([P, st, E], F32, tag="aem")
                    dma(m_t[:], M_in[rows, :].rearrange("(p t) e -> p t e", p=P))
                    dma(trm_t[:], TR_m[rows, :].rearrange("(p t) e -> p t e", p=P))
                    dma(ais_t[:], A_is[rows, :].rearrange("(p t) e -> p t e", p=P))
                    dma(aem_t[:], A_em[rows, :].rearrange("(p t) e -> p t e", p=P))
                    # W(rep 0) rides just behind the first tile's inputs;
                    # later reps prefetch W chunk-pairs interleaved into the
                    # previous rep's last supertiles so the rep boundary
                    # never stalls on the 4 MiB W blob.
                    if s == 0 and _rep == 0:
                        load_w_chunks(w_pair, 0, KT)
                    if _rep + 1 < repeat and s >= nst - 4 and nst >= 4:
                        if w_next is None:
                            w_next = w_tile()
                        q = s - (nst - 4)
                        load_w_chunks(w_next, q * 2, q * 2 + 2, wc=2)

                    # D = A_is - A_em on GPSIMD (otherwise idle), off the
                    # DVE critical path; mc' = (2/3) m + tr_m on DVE.
                    d_t = dpool.tile([P, st, E], F32, tag="d")
                    nc.gpsimd.tensor_tensor(
                        d_t[:], ais_t[:], aem_t[:], AX.subtract
                    )
                    mc_t = dpool.tile([P, st, E], TDT, tag="mc")
                    nc.vector.scalar_tensor_tensor(
                        out=mc_t[:],
                        in0=m_t[:],
                        scalar=2.0 / 3.0,
                        in1=trm_t[:],
                        op0=AX.mult,
                        op1=AX.add,
                    )
                    emit_ttr()

                    for t in range(st):
                        # mc'^T chunks via PE identity transpose (fp32r, 1.5
                        # cycles/row); group copies PSUM->SBUF ride on ACT.
                        mct_t = mctpool.tile([P, KT, P], F8 if FP8 else DT, tag="mct")
                        for g in range(KT // 4):
                            pt = pst.tile([P, 4, P], TDT, tag="pt")
                            for j4 in range(4):
                                j = g * 4 + j4
                                cols = bass.ds(j * P, P)
                                nc.tensor.matmul(
                                    pt[:, j4],
                                    mc_t[:, t, cols],
                                    ident1[:],
                                    is_transpose=True,
                                    start=True,
                                    stop=True,
                                )
                            nc.scalar.copy(
                                mct_t[:, bass.ds(g * 4, 4), :], pt[:]
                            )

                        # P = mc' @ W (fp32r full rate); rowdot deferred
                        pms = []
                        for n in range(NCH):
                            pms.append(
                                psmm.tile([P, NF], F32, tag="pm", name="pm")
                            )
                        if FP8:
                            for k2 in range(KT // 2):
                                kk = bass.ds(2 * k2, 2)
                                for n in range(NCH):
                                    ncols = bass.ds(n * NF, NF)
                                    nc.tensor.matmul(
                                        pms[n][:],
                                        mct_t[:, kk, :],
                                        w_sb[:, kk, ncols],
                                        start=(k2 == 0),
                                        stop=(k2 == KT // 2 - 1),
                                        perf_mode=mybir.MatmulPerfMode.DoubleRow,
                                    )
                        else:
                            for k in range(KT):
                                for n in range(NCH):
                                    ncols = bass.ds(n * NF, NF)
                                    nc.tensor.matmul(
                                        pms[n][:],
                                        mct_t[:, k, :],
                                        w_sb[:, k, ncols],
                                        start=(k == 0),
                                        stop=(k == KT - 1),
                                    )
                        if t < st - 1:
                            emit_ttr()
                        for n in range(NCH):
                            zi = (s * st + t) * NCH + n
                            pending.append(
                                (pms[n], d_t[:, t, bass.ds(n * NF, NF)], zi)
                            )
                emit_ttr()
                if w_next is not None:
                    w_cur = w_next

                # z'_b = sum of its n-chunk partials; hinge; row-reduce
                zrow = accpool.tile([P, NBT], F32, tag="zrow")
                nc.vector.tensor_tensor(
                    zrow[:],
                    zacc[:].rearrange("p (b n) -> p b n", n=NCH)[:, :, 0],
                    zacc[:].rearrange("p (b n) -> p b n", n=NCH)[:, :, 1],
                    AX.add,
                )
                hrow = accpool.tile([P, NBT], F32, tag="hrow")
                nc.vector.tensor_scalar(
                    hrow[:], zrow[:], 1.0 / 3.0, 0.0, AX.add, AX.max
                )
                hsum = accpool.tile([P, 1], F32, tag="hsum")
                nc.vector.reduce_sum(hsum[:], hrow[:], axis=mybir.AxisListType.X)

                # partition reduce (x0.6 folded into the ones vector)
                fin = psfin.tile([1, 1], F32, tag="fin")
                nc.tensor.matmul(fin[:], hsum[:], ones06[:], start=True, stop=True)
                out_sb = accpool.tile([1, 1], F32, tag="osb")
                nc.any.tensor_copy(out_sb[:], fin[:])
                dma(OUT[:], out_sb[:])

    return nc


def _split_multi_waits(raw: bytes) -> bytes:
    """Split multi-wait instructions into single-wait Drain carriers +
    original: this walrus build allows only one sync wait per instruction."""
    import json as _json

    d = _json.loads(raw)
    for fn in d["functions"]:
        for bb in fn["blocks"]:
            out = []
            for inst in bb["instructions"]:
                si = inst.get("sync_info") or {}
                waits = si.get("on_wait") or []
                if len(waits) > 1:
                    for i, w in enumerate(waits[:-1]):
                        carrier = {
                            "engine": inst["engine"],
                            "ins": [],
                            "name": f"{inst['name']}-sw{i}",
                            "opcode": "Drain",
                            "outs": [],
                            "sync_info": {"on_update": [], "on_wait": [w]},
                        }
                        if "debug" in inst:
                            carrier["debug"] = inst["debug"]
                        out.append(carrier)
                    inst["sync_info"] = {
                        "on_update": si.get("on_update") or [],
                        "on_wait": [waits[-1]],
                    }
                out.append(inst)
            bb["instructions"] = out
    return _json.dumps(d).encode()


def _patch_nc(nc):
    patched = _split_multi_waits(nc.to_json_bytes())
    nc.to_json_bytes = lambda: patched
    return nc


_NC_CACHE = None


def _get_nc():
    global _NC_CACHE
    if _NC_CACHE is None:
        _NC_CACHE = _patch_nc(build())
    return _NC_CACHE


def _in_maps(inputs):
    a_is = np.ascontiguousarray(np.asarray(inputs["A_is_t"], dtype=np.float32))
    a_em = np.ascontiguousarray(np.asarray(inputs["A_em_t"], dtype=np.float32))
    m = np.ascontiguousarray(np.asarray(inputs["m"], dtype=np.float32))
    tr_m = np.ascontiguousarray(np.asarray(inputs["tr_m"], dtype=np.float32))
    w = np.ascontiguousarray(np.asarray(inputs["W"], dtype=np.float32))
    maps = []
    for c in range(N_CORES):
        sl = slice(c * B_LOC, (c + 1) * B_LOC)
        maps.append(
            {
                "a_is": a_is[sl],
                "a_em": a_em[sl],
                "m_in": m[sl],
                "tr_m": tr_m[sl],
                "w_in": w,
            }
        )
    return maps


def run(inputs, trace=False, **kw):
    """Run on all 8 cores; returns (full_output, BassKernelResults)."""
    nc = _get_nc()
    res = run_bass_kernel_spmd(
        nc, _in_maps(inputs), list(range(N_CORES)), trace=trace, **kw
    )
    total = float(sum(np.float32(r["out"][0, 0]) for r in res.results))
    return np.array([total], dtype=np.float32), res


def kernel(**inputs) -> np.ndarray:
    out, _ = run(inputs, trace=False)
    return out

